# revision 37
# baseline (speedup 1.0000x reference)
"""Multi-head self-attention (B=2, N=2048, D=1024, H=16, Dh=64) on 8 TRN2 NeuronCores.

Sharding: core c handles batch b = c // 4 and head group g = c % 4 (heads 4g..4g+3).
Tensor-parallel on heads for qkv/out_proj; data-parallel on batch. Each core
produces a partial [D, N] output (transposed, bf16); host sums the 4 head-group
partials per batch, transposes, and adds b_out.

Fully-fused single-region schedule: the softmax exp on the scalar (ACT) engine
is the pacing resource (~131us of exp at 1.2 GHz, dtype-independent), so the
qkv-projection, v-projection and out-projection matmul chains are emitted as
fillers between attention iterations. That keeps the PE warm (no HAM
re-throttle) and hides phases A/C entirely behind the ACT-bound attention loop.
All SBUF tensors are bf16 (PE rate is identical to f32r, DMA bytes halve);
PSUM accumulation stays fp32.
"""
import sys
import numpy as np
import ml_dtypes

for _p in ("/opt/trn_rl_repo", "/root/.axon_site/_ro/trn_rl_repo"):
    if _p not in sys.path:
        sys.path.append(_p)

import concourse.bass as bass
import concourse.bacc as bacc
import concourse.tile as tile
from concourse import mybir
from concourse.bass_utils import run_bass_kernel_spmd

F32 = mybir.dt.float32
BF16 = mybir.dt.bfloat16
EXP = mybir.ActivationFunctionType.Exp
MULT = mybir.AluOpType.mult
ADD = mybir.AluOpType.add

B, S, D = 2, 2048, 1024
H, DH = 16, 64
HL = 4            # heads per core (local)
CQK = 512         # k+q channels per core (2*HL*DH); dram col order [k0 k1 q0 q1]
CV = 256          # v channels per core (HL*DH)
ND = D // 128     # 8 d-tiles
NKT = S // 128    # 16 key tiles
NQC = S // 512    # 4 query blocks of 512


def build_kernel() -> "bass.Bass":
    nc = bacc.Bacc(None, target_bir_lowering=False, debug=False)

    xT = nc.dram_tensor("xT", [D, S], BF16, kind="ExternalInput")
    wqk = nc.dram_tensor("wqk", [D, CQK], BF16, kind="ExternalInput")
    bqk = nc.dram_tensor("bqk", [128, CQK // 128], F32, kind="ExternalInput")
    wv = nc.dram_tensor("wv", [D, CV], BF16, kind="ExternalInput")
    bvb = nc.dram_tensor("bvb", [128, CV], F32, kind="ExternalInput")
    wout = nc.dram_tensor("wout", [CV, D], BF16, kind="ExternalInput")
    outT = nc.dram_tensor("outT", [D, S], BF16, kind="ExternalOutput")

    xT_r = xT.rearrange("(t p) s -> t p s", p=128)        # [8, 128, 2048]
    wqk_r = wqk.rearrange("(t p) c -> t p c", p=128)      # [8, 128, 512]
    wv_r = wv.rearrange("(t p) c -> t p c", p=128)        # [8, 128, 256]
    wout_r = wout.rearrange("(t p) n -> t p n", p=128)    # [2, 128, 1024]
    outT_r = outT.rearrange("(t p) s -> t p s", p=128)    # [8, 128, 2048]

    with tile.TileContext(nc) as tc:
        ctxs = [
            tc.tile_pool(name="persist", bufs=1),
            tc.tile_pool(name="ptp", bufs=8),
            tc.tile_pool(name="small", bufs=3),
            tc.tile_pool(name="stage", bufs=4),
            tc.tile_pool(name="psB", bufs=2, space="PSUM"),
            tc.tile_pool(name="psAV", bufs=1, space="PSUM"),
            tc.tile_pool(name="psF", bufs=2, space="PSUM"),
        ]
        persist, ptp, small, stage, psb, psav, psf = [c.__enter__() for c in ctxs]

        xt_lo = persist.tile([128, ND, S // 2], BF16)   # seq 0:1024
        xt_hi = persist.tile([128, ND, S // 2], BF16)   # seq 1024:2048
        wqk_s = persist.tile([128, ND, CQK], BF16)
        wv_s = persist.tile([128, ND, CV], BF16)
        wout_s = persist.tile([128, 2, D], BF16)
        qkt_s = persist.tile([128, 4, S], BF16)           # m: k0 k1 q0 q1
        v_s = persist.tile([128, NKT, HL, DH + 1], BF16)  # per key-tile V + ones col
        at_s = persist.tile([128, 2, S], BF16)            # normalized attn out^T
        cpart_s = persist.tile([128, ND, 512], F32)       # qb3 out-proj partials
        bqk_s = persist.tile([128, CQK // 128], F32)
        bvb_s = persist.tile([128, CV], F32)

        # ---------------- input DMAs --------------------------------------
        # NOTE: dma_start must stay on nc.sync -- posts from gpsimd/scalar
        # are not correctly awaited by consumers (first-run garbage reads).
        # Critical path first: per-d (wqk, x[0:1024]) pairs feed the first
        # two projection chains; non-critical tensors are merged into single
        # gather-DMAs to cut the ~650ns/post serialization.
        # per-DMA-engine bandwidth is only ~30 GB/s (256KB = ~8.5us), and each
        # engine drains its posts serially -- so the first-exp critical set
        # (bqk, wqk, x[0:1024]) is posted first in small per-d pieces, and
        # everything else queues behind it
        # all posts on the sync queue: it has the fastest HWDGE descriptor
        # rate (~224 GB/s; gpsimd/scalar queues measured slower). Critical
        # first-exp set (bqk, wqk, x[0:1024]) first, in per-d pieces so the
        # ~30 GB/s per-DMA-engine drains stay balanced; the rest queues behind.
        # x tiles (256KB, 8.5us/engine drain) post before wqk (128KB, 4.3us):
        # the PE's steady stream starts at last-critical-arrival, and the
        # slow-draining tiles must enter their engines first
        nc.vector.memset(v_s[:, :, :, DH:DH + 1], 1.0)
        nc.sync.dma_start(out=bqk_s[:], in_=bqk[:])
        for d in range(ND):
            nc.sync.dma_start(out=xt_lo[:, d, :], in_=xT_r[d][:, 0:1024])
        for d in range(ND):
            nc.sync.dma_start(out=wqk_s[:, d, :], in_=wqk_r[d])
        for d in range(ND):
            nc.sync.dma_start(out=wv_s[:, d, :], in_=wv_r[d])
        nc.sync.dma_start(out=bvb_s[:], in_=bvb[:])
        for d in range(ND):
            nc.sync.dma_start(out=xt_hi[:, d, :], in_=xT_r[d][:, 1024:2048])
        nc.sync.dma_start(out=wout_s[:], in_=wout.rearrange("(t p) n -> p t n", p=128))

        def xt_seq(d, lo, width):
            # view of x^T [d-tile, seq lo:lo+width] across the two half-tiles
            if lo + width <= 1024:
                return xt_lo[:, d, lo:lo + width]
            return xt_hi[:, d, lo - 1024:lo - 1024 + width]

        # ---------------- chain builders (each is one PE filler unit) ------
        def a1_chain(m, n):
            # qkt_s[:, m, n*512:(n+1)*512] = wqk_m^T @ x_chunk + bias
            ps = psf.tile([128, 512], F32, tag="fill", name=f"a1_{m}_{n}")
            for d in range(ND):
                nc.tensor.matmul(ps[:], wqk_s[:, d, m * 128:(m + 1) * 128],
                                 xt_seq(d, n * 512, 512),
                                 start=(d == 0), stop=(d == ND - 1))
            nc.vector.tensor_scalar_add(
                qkt_s[:, m, n * 512:(n + 1) * 512], ps[:], bqk_s[:, m:m + 1])

        def a2_chain(st):
            # v_s[:, st] = (x_tile^T @ wv) + bias   (keys on partitions)
            ps = psf.tile([128, CV], F32, tag="fill", name=f"a2_{st}")
            for d in range(ND):
                nc.tensor.matmul(ps[:], xt_seq(d, st * 128, 128),
                                 wv_s[:, d, :],
                                 start=(d == 0), stop=(d == ND - 1))
            nc.vector.tensor_tensor(
                out=v_s[:, st, :, 0:DH],
                in0=ps[:].rearrange("p (h c) -> p h c", h=HL),
                in1=bvb_s[:].rearrange("p (h c) -> p h c", h=HL),
                op=ADD)

        def c_chain(qc, nt):
            # outT[nt, qc-block] = wout^T @ at  (contract local 256 channels)
            qg = slice(qc * 512, (qc + 1) * 512)
            ps = psf.tile([128, 512], F32, tag="fill", name=f"c_{qc}_{nt}")
            for ct in range(2):
                nc.tensor.matmul(ps[:], wout_s[:, ct, nt * 128:(nt + 1) * 128],
                                 at_s[:, ct, qg],
                                 start=(ct == 0), stop=(ct == 1))
            o = stage.tile([128, 512], BF16, tag="o", name="o")
            nc.vector.tensor_copy(out=o[:], in_=ps[:])
            nc.sync.dma_start(out=outT_r[nt][:, qg], in_=o[:])

        # split variant for the last query block: the ct=0 half only needs
        # head-pair 0's normalized output, so it runs as filler during the
        # (qb3, p=1) attention block; ct=1 + combine form the epilogue
        def c3_part0(nt):
            qg = slice(3 * 512, 4 * 512)
            ps = psf.tile([128, 512], F32, tag="fill", name=f"c3a_{nt}")
            nc.tensor.matmul(ps[:], wout_s[:, 0, nt * 128:(nt + 1) * 128],
                             at_s[:, 0, qg], start=True, stop=True)
            nc.vector.tensor_copy(out=cpart_s[:, nt, :], in_=ps[:])

        def c3_part1(nt):
            qg = slice(3 * 512, 4 * 512)
            ps = psf.tile([128, 512], F32, tag="fill", name=f"c3b_{nt}")
            nc.tensor.matmul(ps[:], wout_s[:, 1, nt * 128:(nt + 1) * 128],
                             at_s[:, 1, qg], start=True, stop=True)
            o = stage.tile([128, 512], BF16, tag="o", name="o")
            nc.vector.tensor_tensor(out=o[:], in0=ps[:], in1=cpart_s[:, nt, :],
                                    op=ADD)
            nc.sync.dma_start(out=outT_r[nt][:, qg], in_=o[:])

        # ---------------- attention block with interleaved fillers ---------
        def b_block(qb, p, fillers):
            kt = qkt_s[:, p, :]
            qt = qkt_s[:, 2 + p, :]
            q0 = qb * 512
            qs = slice(q0, q0 + 512)
            pA = psav.tile([DH + 1, 512], F32, tag="pA", name="pA")
            pB = psav.tile([DH + 1, 512], F32, tag="pB", name="pB")
            nf = len(fillers)
            fi = 0
            for t in range(NKT):
                sAB = psb.tile([128, 1024], F32, tag="sAB", name="sAB")
                nc.tensor.matmul(sAB[:, 0:512],
                                 kt[0:64, t * 128:(t + 1) * 128],
                                 qt[0:64, qs], start=True, stop=True,
                                 tile_position=(0, 0))
                nc.tensor.matmul(sAB[:, 512:1024],
                                 kt[64:128, t * 128:(t + 1) * 128],
                                 qt[64:128, qs], start=True, stop=True,
                                 tile_position=(64, 0))
                pt = ptp.tile([128, 1024], BF16, tag="pt", name="pt")
                nc.scalar.activation(pt[:], sAB[:], EXP)
                # fillers sit between exp(t) and PV(t): Tile's dependency
                # tracking is emission-order-semantic, so a producer chain
                # (e.g. the v-projection feeding this block's own PV) must
                # precede its consumer, while scores/exp stay ahead of any
                # DMA-gated filler in the PE FIFO
                want = (t + 1) * nf // NKT
                while fi < want:
                    fillers[fi]()
                    fi += 1
                nc.tensor.matmul(pA[:], v_s[:, t, 2 * p, :],
                                 pt[:, 0:512],
                                 start=(t == 0), stop=(t == NKT - 1))
                nc.tensor.matmul(pB[:], v_s[:, t, 2 * p + 1, :],
                                 pt[:, 512:1024],
                                 start=(t == 0), stop=(t == NKT - 1))
            # normalize by softmax denominator (ones-row of each psum);
            # the [64,8] DMA reshape spreads the reciprocal across partitions
            # (a [1,512] reciprocal costs 3.3us on DVE, [64,8] costs 0.2us)
            for loc, pX in ((0, pA), (1, pB)):
                raw = small.tile([DH + 1, 512], F32, tag="raw", name="raw")
                nc.vector.tensor_copy(out=raw[:], in_=pX[:])
                dn = small.tile([64, 8], F32, tag="dn", name="dn")
                nc.sync.dma_start(out=dn[:], in_=raw[DH:DH + 1, :])
                rr = small.tile([64, 8], F32, tag="rr", name="rr")
                nc.vector.reciprocal(rr[:], dn[:])
                r = small.tile([1, 512], F32, tag="r", name="r")
                nc.sync.dma_start(out=r[:], in_=rr[:])
                rb = small.tile([64, 512], F32, tag="rb", name="rb")
                nc.gpsimd.partition_broadcast(rb[:], r[:])
                if loc == 0:
                    nc.vector.tensor_tensor(
                        out=at_s[0:64, p, qs],
                        in0=raw[0:DH, :], in1=rb[:], op=MULT)
                else:
                    # DVE lanes cannot shift partitions; bounce via DMA
                    tmp = small.tile([64, 512], BF16, tag="tmp", name="tmp")
                    nc.vector.tensor_tensor(
                        out=tmp[:], in0=raw[0:DH, :], in1=rb[:], op=MULT)
                    nc.sync.dma_start(out=at_s[64:128, p, qs], in_=tmp[:])

        # ---------------- prologue ------------------------------------------
        # warm the HAM clock with throwaway matmuls on a locally-initialized
        # tile while x streams in (otherwise the prologue runs at 1.2 GHz)
        warm_src = persist.tile([128, 512], BF16)
        nc.vector.memset(warm_src[:], 0.5)
        for w in range(18):
            pw = psf.tile([128, 512], F32, tag="fill", name=f"warm{w}")
            nc.tensor.matmul(pw[:], warm_src[:, 0:128], warm_src[:],
                             start=True, stop=True)
        # k-p0 (first two seq chunks) + q-p0 (qb0) + first v tiles;
        # everything else is filler
        a1_chain(0, 0)
        a1_chain(2, 0)
        a1_chain(0, 1)
        for st in range(6):
            a2_chain(st)

        # ---------------- fused main loop ----------------------------------
        def F(fn, *a):
            return lambda: fn(*a)

        plan = {
            (0, 0): [F(a2_chain, 6), F(a2_chain, 7), F(a1_chain, 0, 2),
                     F(a2_chain, 8), F(a2_chain, 9), F(a2_chain, 10),
                     F(a2_chain, 11), F(a1_chain, 0, 3), F(a2_chain, 12),
                     F(a2_chain, 13), F(a2_chain, 14), F(a2_chain, 15),
                     F(a1_chain, 1, 0), F(a1_chain, 3, 0)],
            (0, 1): [F(a1_chain, 1, 1), F(a1_chain, 1, 2), F(a1_chain, 1, 3),
                     F(a1_chain, 2, 1), F(a1_chain, 3, 1)],
            (1, 0): [F(c_chain, 0, nt) for nt in range(5)]
                    + [F(a1_chain, 2, 2), F(a1_chain, 3, 2)],
            (1, 1): [F(c_chain, 0, nt) for nt in range(5, ND)]
                    + [F(a1_chain, 2, 3), F(a1_chain, 3, 3)],
            (2, 0): [F(c_chain, 1, nt) for nt in range(5)],
            (2, 1): [F(c_chain, 1, nt) for nt in range(5, ND)],
            (3, 0): [F(c_chain, 2, nt) for nt in range(5)],
            (3, 1): [F(c_chain, 2, nt) for nt in range(5, ND)]
                    + [F(c3_part0, nt) for nt in range(ND)],
        }
        for qb in range(NQC):
            for p in range(2):
                b_block(qb, p, plan[(qb, p)])
        # epilogue: keep the PE (and its HAM clock) busy with throwaway
        # matmuls while the final softmax-normalize DMA chain drains, then
        # finish the last out-projection block at full clock
        for w in range(44):
            pw = psf.tile([128, 512], F32, tag="fill", name=f"ewarm{w}")
            nc.tensor.matmul(pw[:], warm_src[:, 0:128], warm_src[:],
                             start=True, stop=True)
        for nt in range(ND):
            c3_part1(nt)

        for c in reversed(ctxs):
            c.__exit__(None, None, None)
    nc.compile()
    return nc


def shard_inputs(x, W_qkv, b_qkv, W_out, b_out=None):
    """Build the 8 per-core input maps. Core c: batch c//4, head group c%4."""
    in_maps = []
    scale = 1.0 / np.sqrt(np.float32(DH))
    bf16 = ml_dtypes.bfloat16
    for c in range(8):
        b, g = divmod(c, 4)
        cs = slice(g * 256, g * 256 + 256)
        xTc = np.ascontiguousarray(x[b].T)                       # [D, S]
        wq = W_qkv[:, 0:D][:, cs] * scale                        # [D, 256]
        wk = W_qkv[:, D:2 * D][:, cs]
        wqkc = np.ascontiguousarray(np.concatenate([wk, wq], axis=1))  # [D, 512] k first
        bq = b_qkv[0:D][cs] * scale
        bk = b_qkv[D:2 * D][cs]
        bqkc = np.concatenate([bk, bq]).reshape(CQK // 128, 128).T     # [128, 4]
        bqkc = np.ascontiguousarray(bqkc)
        wvc = np.ascontiguousarray(W_qkv[:, 2 * D:3 * D][:, cs])       # [D, 256]
        bvbc = np.ascontiguousarray(
            np.broadcast_to(b_qkv[2 * D:3 * D][cs], (128, CV)))        # [128, 256]
        woutc = np.ascontiguousarray(W_out[cs, :])                     # [256, D]
        in_maps.append({
            "xT": xTc.astype(bf16),
            "wqk": wqkc.astype(bf16),
            "bqk": bqkc.astype(np.float32),
            "wv": wvc.astype(bf16),
            "bvb": bvbc.astype(np.float32),
            "wout": woutc.astype(bf16),
        })
    return in_maps


_NC_CACHE = []


def _get_nc():
    if not _NC_CACHE:
        _NC_CACHE.append(build_kernel())
    return _NC_CACHE[0]


def run_sharded(in_maps, **kwargs):
    nc = _get_nc()
    return run_bass_kernel_spmd(nc, in_maps, core_ids=list(range(8)), **kwargs)


def gather_output(results, b_out):
    out = np.empty((B, S, D), dtype=np.float32)
    for b in range(B):
        acc = results[4 * b]["outT"].astype(np.float32)
        for g in range(1, 4):
            acc = acc + results[4 * b + g]["outT"].astype(np.float32)
        out[b] = acc.T + b_out[None, :]
    return out


def kernel(x, W_qkv, b_qkv, W_out, b_out):
    x = np.asarray(x, dtype=np.float32)
    W_qkv = np.asarray(W_qkv, dtype=np.float32)
    b_qkv = np.asarray(b_qkv, dtype=np.float32)
    W_out = np.asarray(W_out, dtype=np.float32)
    b_out = np.asarray(b_out, dtype=np.float32)
    in_maps = shard_inputs(x=x, W_qkv=W_qkv, b_qkv=b_qkv, W_out=W_out, b_out=b_out)
    res = run_sharded(in_maps)
    return gather_output(res.results, b_out)


# revision 38
# speedup vs baseline: 1.0094x; 1.0094x over previous
"""Multi-head self-attention (B=2, N=2048, D=1024, H=16, Dh=64) on 8 TRN2 NeuronCores.

Sharding: core c handles batch b = c // 4 and head group g = c % 4 (heads 4g..4g+3).
Tensor-parallel on heads for qkv/out_proj; data-parallel on batch. Each core
produces a partial [D, N] output (transposed, bf16); host sums the 4 head-group
partials per batch, transposes, and adds b_out.

Fully-fused single-region schedule: the softmax exp on the scalar (ACT) engine
is the pacing resource (~131us of exp at 1.2 GHz, dtype-independent), so the
qkv-projection, v-projection and out-projection matmul chains are emitted as
fillers between attention iterations. That keeps the PE warm (no HAM
re-throttle) and hides phases A/C entirely behind the ACT-bound attention loop.
All SBUF tensors are bf16 (PE rate is identical to f32r, DMA bytes halve);
PSUM accumulation stays fp32.
"""
import sys
import numpy as np
import ml_dtypes

for _p in ("/opt/trn_rl_repo", "/root/.axon_site/_ro/trn_rl_repo"):
    if _p not in sys.path:
        sys.path.append(_p)

import concourse.bass as bass
import concourse.bacc as bacc
import concourse.tile as tile
from concourse import mybir
from concourse.bass_utils import run_bass_kernel_spmd

F32 = mybir.dt.float32
BF16 = mybir.dt.bfloat16
EXP = mybir.ActivationFunctionType.Exp
MULT = mybir.AluOpType.mult
ADD = mybir.AluOpType.add

B, S, D = 2, 2048, 1024
H, DH = 16, 64
HL = 4            # heads per core (local)
CQK = 512         # k+q channels per core (2*HL*DH); dram col order [k0 k1 q0 q1]
CV = 256          # v channels per core (HL*DH)
ND = D // 128     # 8 d-tiles
NKT = S // 128    # 16 key tiles
NQC = S // 512    # 4 query blocks of 512


def build_kernel() -> "bass.Bass":
    nc = bacc.Bacc(None, target_bir_lowering=False, debug=False)

    xT = nc.dram_tensor("xT", [D, S], BF16, kind="ExternalInput")
    wqk = nc.dram_tensor("wqk", [D, CQK], BF16, kind="ExternalInput")
    bqk = nc.dram_tensor("bqk", [128, CQK // 128], F32, kind="ExternalInput")
    wv = nc.dram_tensor("wv", [D, CV], BF16, kind="ExternalInput")
    bvb = nc.dram_tensor("bvb", [128, CV], F32, kind="ExternalInput")
    wout = nc.dram_tensor("wout", [CV, D], BF16, kind="ExternalInput")
    outT = nc.dram_tensor("outT", [D, S], BF16, kind="ExternalOutput")

    xT_r = xT.rearrange("(t p) s -> t p s", p=128)        # [8, 128, 2048]
    wqk_r = wqk.rearrange("(t p) c -> t p c", p=128)      # [8, 128, 512]
    wv_r = wv.rearrange("(t p) c -> t p c", p=128)        # [8, 128, 256]
    wout_r = wout.rearrange("(t p) n -> t p n", p=128)    # [2, 128, 1024]
    outT_r = outT.rearrange("(t p) s -> t p s", p=128)    # [8, 128, 2048]

    with tile.TileContext(nc) as tc:
        ctxs = [
            tc.tile_pool(name="persist", bufs=1),
            tc.tile_pool(name="ptp", bufs=8),
            tc.tile_pool(name="small", bufs=3),
            tc.tile_pool(name="stage", bufs=4),
            tc.tile_pool(name="psB", bufs=2, space="PSUM"),
            tc.tile_pool(name="psAV", bufs=1, space="PSUM"),
            tc.tile_pool(name="psF", bufs=2, space="PSUM"),
        ]
        persist, ptp, small, stage, psb, psav, psf = [c.__enter__() for c in ctxs]

        xt_lo = persist.tile([128, ND, S // 2], BF16)   # seq 0:1024
        xt_hi = persist.tile([128, ND, S // 2], BF16)   # seq 1024:2048
        wqk_s = persist.tile([128, ND, CQK], BF16)
        wv_s = persist.tile([128, ND, CV], BF16)
        wout_s = persist.tile([128, 2, D], BF16)
        qkt_s = persist.tile([128, 4, S], BF16)           # m: k0 k1 q0 q1
        v_s = persist.tile([128, NKT, HL, DH + 1], BF16)  # per key-tile V + ones col
        at_s = persist.tile([128, 2, S], BF16)            # normalized attn out^T
        cpart_s = persist.tile([128, ND, 512], F32)       # qb3 out-proj partials
        bqk_s = persist.tile([128, CQK // 128], F32)
        bvb_s = persist.tile([128, CV], F32)

        # ---------------- input DMAs --------------------------------------
        # NOTE: dma_start must stay on nc.sync -- posts from gpsimd/scalar
        # are not correctly awaited by consumers (first-run garbage reads).
        # Critical path first: per-d (wqk, x[0:1024]) pairs feed the first
        # two projection chains; non-critical tensors are merged into single
        # gather-DMAs to cut the ~650ns/post serialization.
        # per-DMA-engine bandwidth is only ~30 GB/s (256KB = ~8.5us), and each
        # engine drains its posts serially -- so the first-exp critical set
        # (bqk, wqk, x[0:1024]) is posted first in small per-d pieces, and
        # everything else queues behind it
        # all posts on the sync queue: it has the fastest HWDGE descriptor
        # rate (~224 GB/s; gpsimd/scalar queues measured slower). Critical
        # first-exp set (bqk, wqk, x[0:1024]) first, in per-d pieces so the
        # ~30 GB/s per-DMA-engine drains stay balanced; the rest queues behind.
        # x tiles (256KB, 8.5us/engine drain) post before wqk (128KB, 4.3us):
        # the PE's steady stream starts at last-critical-arrival, and the
        # slow-draining tiles must enter their engines first
        nc.vector.memset(v_s[:, :, :, DH:DH + 1], 1.0)
        nc.sync.dma_start(out=bqk_s[:], in_=bqk[:])
        for d in range(ND):
            nc.sync.dma_start(out=xt_lo[:, d, :], in_=xT_r[d][:, 0:1024])
        for d in range(ND):
            nc.sync.dma_start(out=wqk_s[:, d, :], in_=wqk_r[d])
        for d in range(ND):
            nc.sync.dma_start(out=wv_s[:, d, :], in_=wv_r[d])
        nc.sync.dma_start(out=bvb_s[:], in_=bvb[:])
        for d in range(ND):
            nc.sync.dma_start(out=xt_hi[:, d, :], in_=xT_r[d][:, 1024:2048])
        nc.sync.dma_start(out=wout_s[:], in_=wout.rearrange("(t p) n -> p t n", p=128))

        def xt_seq(d, lo, width):
            # view of x^T [d-tile, seq lo:lo+width] across the two half-tiles
            if lo + width <= 1024:
                return xt_lo[:, d, lo:lo + width]
            return xt_hi[:, d, lo - 1024:lo - 1024 + width]

        # ---------------- chain builders (each is one PE filler unit) ------
        def a1_chain(m, n):
            # qkt_s[:, m, n*512:(n+1)*512] = wqk_m^T @ x_chunk + bias
            ps = psf.tile([128, 512], F32, tag="fill", name=f"a1_{m}_{n}")
            for d in range(ND):
                nc.tensor.matmul(ps[:], wqk_s[:, d, m * 128:(m + 1) * 128],
                                 xt_seq(d, n * 512, 512),
                                 start=(d == 0), stop=(d == ND - 1))
            nc.vector.tensor_scalar_add(
                qkt_s[:, m, n * 512:(n + 1) * 512], ps[:], bqk_s[:, m:m + 1])

        def a2_chain(st):
            # v_s[:, st] = (x_tile^T @ wv) + bias   (keys on partitions)
            ps = psf.tile([128, CV], F32, tag="fill", name=f"a2_{st}")
            for d in range(ND):
                nc.tensor.matmul(ps[:], xt_seq(d, st * 128, 128),
                                 wv_s[:, d, :],
                                 start=(d == 0), stop=(d == ND - 1))
            nc.vector.tensor_tensor(
                out=v_s[:, st, :, 0:DH],
                in0=ps[:].rearrange("p (h c) -> p h c", h=HL),
                in1=bvb_s[:].rearrange("p (h c) -> p h c", h=HL),
                op=ADD)

        def c_chain(qc, nt):
            # outT[nt, qc-block] = wout^T @ at  (contract local 256 channels)
            qg = slice(qc * 512, (qc + 1) * 512)
            ps = psf.tile([128, 512], F32, tag="fill", name=f"c_{qc}_{nt}")
            for ct in range(2):
                nc.tensor.matmul(ps[:], wout_s[:, ct, nt * 128:(nt + 1) * 128],
                                 at_s[:, ct, qg],
                                 start=(ct == 0), stop=(ct == 1))
            o = stage.tile([128, 512], BF16, tag="o", name="o")
            nc.vector.tensor_copy(out=o[:], in_=ps[:])
            nc.sync.dma_start(out=outT_r[nt][:, qg], in_=o[:])

        # split variant for the last query block: the ct=0 half only needs
        # head-pair 0's normalized output, so it runs as filler during the
        # (qb3, p=1) attention block; ct=1 + combine form the epilogue
        def c3_part0(nt):
            qg = slice(3 * 512, 4 * 512)
            ps = psf.tile([128, 512], F32, tag="fill", name=f"c3a_{nt}")
            nc.tensor.matmul(ps[:], wout_s[:, 0, nt * 128:(nt + 1) * 128],
                             at_s[:, 0, qg], start=True, stop=True)
            nc.vector.tensor_copy(out=cpart_s[:, nt, :], in_=ps[:])

        def c3_part1(nt):
            qg = slice(3 * 512, 4 * 512)
            ps = psf.tile([128, 512], F32, tag="fill", name=f"c3b_{nt}")
            nc.tensor.matmul(ps[:], wout_s[:, 1, nt * 128:(nt + 1) * 128],
                             at_s[:, 1, qg], start=True, stop=True)
            o = stage.tile([128, 512], BF16, tag="o", name="o")
            nc.vector.tensor_tensor(out=o[:], in0=ps[:], in1=cpart_s[:, nt, :],
                                    op=ADD)
            nc.sync.dma_start(out=outT_r[nt][:, qg], in_=o[:])

        # ---------------- attention block with interleaved fillers ---------
        def b_block(qb, p, fillers):
            kt = qkt_s[:, p, :]
            qt = qkt_s[:, 2 + p, :]
            q0 = qb * 512
            qs = slice(q0, q0 + 512)
            pA = psav.tile([DH + 1, 512], F32, tag="pA", name="pA")
            pB = psav.tile([DH + 1, 512], F32, tag="pB", name="pB")
            nf = len(fillers)
            fi = 0
            for t in range(NKT):
                sAB = psb.tile([128, 1024], F32, tag="sAB", name="sAB")
                nc.tensor.matmul(sAB[:, 0:512],
                                 kt[0:64, t * 128:(t + 1) * 128],
                                 qt[0:64, qs], start=True, stop=True,
                                 tile_position=(0, 0))
                nc.tensor.matmul(sAB[:, 512:1024],
                                 kt[64:128, t * 128:(t + 1) * 128],
                                 qt[64:128, qs], start=True, stop=True,
                                 tile_position=(64, 0))
                pt = ptp.tile([128, 1024], BF16, tag="pt", name="pt")
                nc.scalar.activation(pt[:], sAB[:], EXP)
                # fillers sit between exp(t) and PV(t): Tile's dependency
                # tracking is emission-order-semantic, so a producer chain
                # (e.g. the v-projection feeding this block's own PV) must
                # precede its consumer, while scores/exp stay ahead of any
                # DMA-gated filler in the PE FIFO
                want = (t + 1) * nf // NKT
                while fi < want:
                    fillers[fi]()
                    fi += 1
                nc.tensor.matmul(pA[:], v_s[:, t, 2 * p, :],
                                 pt[:, 0:512],
                                 start=(t == 0), stop=(t == NKT - 1))
                nc.tensor.matmul(pB[:], v_s[:, t, 2 * p + 1, :],
                                 pt[:, 512:1024],
                                 start=(t == 0), stop=(t == NKT - 1))
            # normalize by softmax denominator (ones-row of each psum);
            # the [64,8] DMA reshape spreads the reciprocal across partitions
            # (a [1,512] reciprocal costs 3.3us on DVE, [64,8] costs 0.2us)
            for loc, pX in ((0, pA), (1, pB)):
                raw = small.tile([DH + 1, 512], F32, tag="raw", name="raw")
                nc.vector.tensor_copy(out=raw[:], in_=pX[:])
                dn = small.tile([64, 8], F32, tag="dn", name="dn")
                nc.sync.dma_start(out=dn[:], in_=raw[DH:DH + 1, :])
                rr = small.tile([64, 8], F32, tag="rr", name="rr")
                nc.vector.reciprocal(rr[:], dn[:])
                r = small.tile([1, 512], F32, tag="r", name="r")
                nc.sync.dma_start(out=r[:], in_=rr[:])
                rb = small.tile([64, 512], F32, tag="rb", name="rb")
                nc.gpsimd.partition_broadcast(rb[:], r[:])
                if loc == 0:
                    nc.vector.tensor_tensor(
                        out=at_s[0:64, p, qs],
                        in0=raw[0:DH, :], in1=rb[:], op=MULT)
                else:
                    # DVE lanes cannot shift partitions; bounce via DMA
                    tmp = small.tile([64, 512], BF16, tag="tmp", name="tmp")
                    nc.vector.tensor_tensor(
                        out=tmp[:], in0=raw[0:DH, :], in1=rb[:], op=MULT)
                    nc.sync.dma_start(out=at_s[64:128, p, qs], in_=tmp[:])

        # ---------------- prologue ------------------------------------------
        # warm the HAM clock with throwaway matmuls on a locally-initialized
        # tile while x streams in (otherwise the prologue runs at 1.2 GHz)
        # enough throwaway matmuls to stay busy until the first projection
        # chain's inputs land (~18us): a >3.4us PE-idle gap here re-throttles
        # the HAM clock and the whole prologue then runs at 1.2 GHz
        warm_src = persist.tile([128, 512], BF16)
        nc.vector.memset(warm_src[:], 0.5)
        for w in range(30):
            pw = psf.tile([128, 512], F32, tag="fill", name=f"warm{w}")
            nc.tensor.matmul(pw[:], warm_src[:, 0:128], warm_src[:],
                             start=True, stop=True)
        # k-p0 (first two seq chunks) + q-p0 (qb0) + first v tiles;
        # everything else is filler
        a1_chain(0, 0)
        a1_chain(2, 0)
        a1_chain(0, 1)
        for st in range(6):
            a2_chain(st)

        # ---------------- fused main loop ----------------------------------
        def F(fn, *a):
            return lambda: fn(*a)

        plan = {
            (0, 0): [F(a2_chain, 6), F(a2_chain, 7), F(a1_chain, 0, 2),
                     F(a2_chain, 8), F(a2_chain, 9), F(a2_chain, 10),
                     F(a2_chain, 11), F(a1_chain, 0, 3), F(a2_chain, 12),
                     F(a2_chain, 13), F(a2_chain, 14), F(a2_chain, 15),
                     F(a1_chain, 1, 0), F(a1_chain, 3, 0)],
            (0, 1): [F(a1_chain, 1, 1), F(a1_chain, 1, 2), F(a1_chain, 1, 3),
                     F(a1_chain, 2, 1), F(a1_chain, 3, 1)],
            (1, 0): [F(c_chain, 0, nt) for nt in range(5)]
                    + [F(a1_chain, 2, 2), F(a1_chain, 3, 2)],
            (1, 1): [F(c_chain, 0, nt) for nt in range(5, ND)]
                    + [F(a1_chain, 2, 3), F(a1_chain, 3, 3)],
            (2, 0): [F(c_chain, 1, nt) for nt in range(5)],
            (2, 1): [F(c_chain, 1, nt) for nt in range(5, ND)],
            (3, 0): [F(c_chain, 2, nt) for nt in range(5)],
            (3, 1): [F(c_chain, 2, nt) for nt in range(5, ND)]
                    + [F(c3_part0, nt) for nt in range(ND)],
        }
        for qb in range(NQC):
            for p in range(2):
                b_block(qb, p, plan[(qb, p)])
        # epilogue: keep the PE (and its HAM clock) busy with throwaway
        # matmuls while the final softmax-normalize DMA chain drains, then
        # finish the last out-projection block at full clock
        for w in range(44):
            pw = psf.tile([128, 512], F32, tag="fill", name=f"ewarm{w}")
            nc.tensor.matmul(pw[:], warm_src[:, 0:128], warm_src[:],
                             start=True, stop=True)
        for nt in range(ND):
            c3_part1(nt)

        for c in reversed(ctxs):
            c.__exit__(None, None, None)
    nc.compile()
    return nc


def shard_inputs(x, W_qkv, b_qkv, W_out, b_out=None):
    """Build the 8 per-core input maps. Core c: batch c//4, head group c%4."""
    in_maps = []
    scale = 1.0 / np.sqrt(np.float32(DH))
    bf16 = ml_dtypes.bfloat16
    for c in range(8):
        b, g = divmod(c, 4)
        cs = slice(g * 256, g * 256 + 256)
        xTc = np.ascontiguousarray(x[b].T)                       # [D, S]
        wq = W_qkv[:, 0:D][:, cs] * scale                        # [D, 256]
        wk = W_qkv[:, D:2 * D][:, cs]
        wqkc = np.ascontiguousarray(np.concatenate([wk, wq], axis=1))  # [D, 512] k first
        bq = b_qkv[0:D][cs] * scale
        bk = b_qkv[D:2 * D][cs]
        bqkc = np.concatenate([bk, bq]).reshape(CQK // 128, 128).T     # [128, 4]
        bqkc = np.ascontiguousarray(bqkc)
        wvc = np.ascontiguousarray(W_qkv[:, 2 * D:3 * D][:, cs])       # [D, 256]
        bvbc = np.ascontiguousarray(
            np.broadcast_to(b_qkv[2 * D:3 * D][cs], (128, CV)))        # [128, 256]
        woutc = np.ascontiguousarray(W_out[cs, :])                     # [256, D]
        in_maps.append({
            "xT": xTc.astype(bf16),
            "wqk": wqkc.astype(bf16),
            "bqk": bqkc.astype(np.float32),
            "wv": wvc.astype(bf16),
            "bvb": bvbc.astype(np.float32),
            "wout": woutc.astype(bf16),
        })
    return in_maps


_NC_CACHE = []


def _get_nc():
    if not _NC_CACHE:
        _NC_CACHE.append(build_kernel())
    return _NC_CACHE[0]


def run_sharded(in_maps, **kwargs):
    nc = _get_nc()
    return run_bass_kernel_spmd(nc, in_maps, core_ids=list(range(8)), **kwargs)


def gather_output(results, b_out):
    out = np.empty((B, S, D), dtype=np.float32)
    for b in range(B):
        acc = results[4 * b]["outT"].astype(np.float32)
        for g in range(1, 4):
            acc = acc + results[4 * b + g]["outT"].astype(np.float32)
        out[b] = acc.T + b_out[None, :]
    return out


def kernel(x, W_qkv, b_qkv, W_out, b_out):
    x = np.asarray(x, dtype=np.float32)
    W_qkv = np.asarray(W_qkv, dtype=np.float32)
    b_qkv = np.asarray(b_qkv, dtype=np.float32)
    W_out = np.asarray(W_out, dtype=np.float32)
    b_out = np.asarray(b_out, dtype=np.float32)
    in_maps = shard_inputs(x=x, W_qkv=W_qkv, b_qkv=b_qkv, W_out=W_out, b_out=b_out)
    res = run_sharded(in_maps)
    return gather_output(res.results, b_out)
